# revision 4
# baseline (speedup 1.0000x reference)
"""DeltaGRU Trainium2 kernel: 2-layer delta-GRU (H=512) over T=1024, B=64.

Strategy: data-parallel over batch across 8 NeuronCores (8 samples/core),
weights replicated. Per core one Bass kernel runs the whole recurrence:

- State kept H-major ("transposed": H on partitions, batch on free dim) so all
  gate math runs on [128, *] tiles.
- Matmuls: mac.T = W @ delta with weight tiles stationary as lhsT. Weights and
  per-tick deltas are split hi/lo into bf16 pairs; each logical fp32 matmul is
  3 bf16 matmuls (Whi@dhi + Whi@dlo + Wlo@dhi) accumulating in fp32 PSUM.
  Per-element error ~5e-6, far below this problem's intrinsic fp32 chaos floor
  (threshold flips amplify any rounding to ~1e-2 maxrel), and ~6x faster than
  fp32 matmuls whose 4-byte stationary-weight load dominates.
- The GRU delta-memory (dm, dm_nh) lives permanently in PSUM; each tick's
  matmuls accumulate into it (start=False); sigmoid/tanh read PSUM directly.
  One PSUM bank is claimed once (start=True) by the bias-init matmuls.
- Input feature expansion (i, q, amp, amp^3, q/amp, i/amp) computed on device
  in a pre-pass; per-tick input slices are dynamic SBUF views - the recurrent
  loop performs zero DMA.
- One For_i dynamic loop over the 1024 timesteps; sigmoid+tanh share one ACT
  table so there are no activation-table reloads inside the loop.

kernel(**inputs) takes the full unsharded inputs, returns [64, 1024, 2] f32.
"""
import numpy as np
import ml_dtypes
import concourse.bass as bass
import concourse.tile as tile
import concourse.mybir as mybir
from concourse import bacc
from concourse.bass_utils import run_bass_kernel_spmd

dt = mybir.dt
Alu = mybir.AluOpType
Act = mybir.ActivationFunctionType

H = 512
KT = H // 128
TH_X = 0.1
TH_H = 0.05
B, T, OUT = 64, 1024, 2
NCORES = 8
BC = B // NCORES


def _bias_layout(bc):
    bo = {}
    cur = 0
    for l in range(2):
        bo[("rz", l)] = cur; cur += 2 * H
        bo[("n", l)] = cur; cur += H
        bo[("nh", l)] = cur; cur += H
    bo["fc"] = cur; cur += 2
    bo["ones"] = cur; cur += bc
    return bo, cur


def _weight_mats(inp):
    return (
        np.ascontiguousarray(np.asarray(inp["W_hh_l0"], np.float32).T),
        np.ascontiguousarray(np.asarray(inp["W_ih_l0"], np.float32).T),
        np.ascontiguousarray(np.asarray(inp["W_ih_l1"], np.float32).T),
        np.ascontiguousarray(np.asarray(inp["W_hh_l1"], np.float32).T),
    )


def _pack_weights_fp(inp, bc):
    """fp32 blob: FC tiles + bias/ones row (hidden weights live in the bf16 blob)."""
    cols = []
    Wfc = np.ascontiguousarray(np.asarray(inp["W_fc"], np.float32).T)
    fc = np.zeros((128, 4 * 2), np.float32)
    for k in range(KT):
        fc[:, k * 2:(k + 1) * 2] = Wfc[k * 128:(k + 1) * 128, :]
    cols.append(fc)

    bo, blen = _bias_layout(bc)
    brow = np.zeros((blen,), np.float32)
    for l in range(2):
        b_ih = np.asarray(inp[f"b_ih_l{l}"], np.float32)
        b_hh = np.asarray(inp[f"b_hh_l{l}"], np.float32)
        brow[bo[("rz", l)]:bo[("rz", l)] + 2 * H] = np.concatenate(
            [b_ih[:H] + b_hh[:H], b_ih[H:2 * H] + b_hh[H:2 * H]])
        brow[bo[("n", l)]:bo[("n", l)] + H] = b_ih[2 * H:]
        brow[bo[("nh", l)]:bo[("nh", l)] + H] = b_hh[2 * H:]
    brow[bo["fc"]:bo["fc"] + 2] = np.asarray(inp["b_fc"], np.float32)
    brow[bo["ones"]:bo["ones"] + bc] = 1.0
    bt = np.zeros((128, blen), np.float32)
    bt[0, :] = brow
    cols.append(bt)
    return np.ascontiguousarray(np.concatenate(cols, axis=1))


def _pack_weights_bf(inp):
    """bf16 hi/lo tile blob; order mirrors _emit's wtb bookkeeping."""
    cols = []

    def tile_pair(mat_T, k0, kn, m0):
        t = np.zeros((128, 128), np.float32)
        t[:kn, :] = mat_T[k0:k0 + kn, m0:m0 + 128]
        hi = t.astype(ml_dtypes.bfloat16)
        lo = (t - hi.astype(np.float32)).astype(ml_dtypes.bfloat16)
        return hi, lo

    Whh0, Wih0, Wih1, Whh1 = _weight_mats(inp)
    for W in (Whh0, Whh1):
        for row0 in (0, H, 2 * H):
            for m in range(4):
                for k in range(KT):
                    hi, lo = tile_pair(W, k * 128, 128, row0 + m * 128)
                    cols.append(hi); cols.append(lo)
    for row0 in (0, H, 2 * H):
        for m in range(4):
            hi, lo = tile_pair(Wih0, 0, 6, row0 + m * 128)
            cols.append(hi); cols.append(lo)
    for row0 in (0, H, 2 * H):
        for m in range(4):
            for k in range(KT):
                hi, lo = tile_pair(Wih1, k * 128, 128, row0 + m * 128)
                cols.append(hi); cols.append(lo)
    return np.ascontiguousarray(np.concatenate(cols, axis=1))


def _wcols_fp(bc):
    _, blen = _bias_layout(bc)
    return 8 + blen


def _wbcols():
    return (2 * 3 * 4 * KT + 3 * 4 + 3 * 4 * KT) * 2 * 128


def _build_kernel(T_, bc, reps=1):
    nc = bacc.Bacc("TRN2", target_bir_lowering=False)
    x_d = nc.dram_tensor("xin", [2, T_ * bc], dt.float32, kind="ExternalInput")
    w_d = nc.dram_tensor("wblob", [128, _wcols_fp(bc)], dt.float32, kind="ExternalInput")
    wb_d = nc.dram_tensor("wbblob", [128, _wbcols()], dt.bfloat16, kind="ExternalInput")
    o_d = nc.dram_tensor("out", [2, T_ * bc], dt.float32, kind="ExternalOutput")
    with tile.TileContext(nc) as tc:
        _emit(nc, tc, x_d, w_d, wb_d, o_d, T_, bc, reps)
    nc.finalize()
    return nc


def _emit(nc, tc, x_d, w_d, wb_d, o_d, T, bc, reps=1):
    import contextlib
    ctx = contextlib.ExitStack()
    sb = ctx.enter_context(tc.tile_pool(name="sb", bufs=1))
    ps = ctx.enter_context(tc.tile_pool(name="ps", bufs=1, space="PSUM"))

    w_s = sb.tile([128, _wcols_fp(bc)], dt.float32, tag="wblob")
    nc.gpsimd.dma_start(w_s[:], w_d[:, :])
    wb_s = sb.tile([128, _wbcols()], dt.bfloat16, tag="wbblob")
    nc.gpsimd.dma_start(wb_s[:], wb_d[:, :])
    feat = sb.tile([8, T * bc], dt.float32, tag="feat")

    fc_tiles = w_s[:, 0:8]
    bias_off = 8
    bo, _ = _bias_layout(bc)

    def bias_ap(start, ln):
        return w_s[0:1, bias_off + start: bias_off + start + ln]

    ones = bias_ap(bo["ones"], bc)

    wtb = {}
    boff = [0]

    def next_btile():
        ap = wb_s[:, boff[0]:boff[0] + 128]
        boff[0] += 128
        return ap

    for l in range(2):
        for g in ("r", "z", "nh"):
            for m in range(4):
                for k in range(KT):
                    wtb[("hh", l, g, m, k)] = (next_btile(), next_btile())
    for g in ("r", "z", "n"):
        for m in range(4):
            wtb[("ih", 0, g, m, 0)] = (next_btile(), next_btile())
    for g in ("r", "z", "n"):
        for m in range(4):
            for k in range(KT):
                wtb[("ih", 1, g, m, k)] = (next_btile(), next_btile())

    h = sb.tile([128, 2 * KT * bc], dt.float32, tag="h")
    hp = sb.tile([128, 2 * KT * bc], dt.float32, tag="hp")
    dh = sb.tile([128, 2 * KT * bc], dt.float32, tag="dh")
    sc = sb.tile([128, 2 * KT * bc], dt.float32, tag="sc")
    xp1 = sb.tile([128, KT * bc], dt.float32, tag="xp1")
    dx1 = sb.tile([128, KT * bc], dt.float32, tag="dx1")
    sc1 = sb.tile([128, KT * bc], dt.float32, tag="sc1")
    xp0 = sb.tile([8, bc], dt.float32, tag="xp0")
    dx0 = sb.tile([8, bc], dt.float32, tag="dx0")
    sc0 = sb.tile([8, bc], dt.float32, tag="sc0")
    rz_s = sb.tile([128, 8 * bc], dt.float32, tag="rzs")
    a_s = sb.tile([128, KT * bc], dt.float32, tag="as")
    b_s = sb.tile([128, KT * bc], dt.float32, tag="bs")
    n_s = sb.tile([128, KT * bc], dt.float32, tag="ns")
    u_s = sb.tile([128, KT * bc], dt.float32, tag="us")
    outring = sb.tile([2, T * bc], dt.float32, tag="outring")
    dh_hi = sb.tile([128, 2 * KT * bc], dt.bfloat16, tag="dhhi")
    dh_lo = sb.tile([128, 2 * KT * bc], dt.bfloat16, tag="dhlo")
    dx1_hi = sb.tile([128, KT * bc], dt.bfloat16, tag="dx1hi")
    dx1_lo = sb.tile([128, KT * bc], dt.bfloat16, tag="dx1lo")
    dx0_hi = sb.tile([8, bc], dt.bfloat16, tag="dx0hi")
    dx0_lo = sb.tile([8, bc], dt.bfloat16, tag="dx0lo")

    def hsl(l):
        return h[:, l * KT * bc:(l + 1) * KT * bc]

    def hks(l, k):
        return h[:, (l * KT + k) * bc:(l * KT + k + 1) * bc]

    ps_rz = []
    for l in range(2):
        t = ps.tile([128, 8 * bc], dt.float32, tag=f"psrz{l}", name=f"psrz{l}")
        ps_rz.append(t)
    ps_n = []
    for l in range(2):
        t = ps.tile([128, 8 * bc], dt.float32, tag=f"psn{l}", name=f"psn{l}")
        ps_n.append(t)
    ps_fc = ps.tile([2, bc], dt.float32, tag="psfc")

    def rz_ps(l, g, m):
        return ps_rz[l][:, (g * 4 + m) * bc:(g * 4 + m + 1) * bc]

    def n_ps(l, g, m):
        return ps_n[l][:, (g * 4 + m) * bc:(g * 4 + m + 1) * bc]

    # feature expansion on [128, N/128] tiles, scattered into feat rows via DMA
    N = T * bc
    FCW = N // 128
    xi = sb.tile([128, FCW], dt.float32, tag="xi")
    xq = sb.tile([128, FCW], dt.float32, tag="xq")
    fs = sb.tile([128, FCW], dt.float32, tag="fs")
    fv = sb.tile([128, FCW], dt.float32, tag="fv")
    famp = sb.tile([128, FCW], dt.float32, tag="famp")
    famp3 = sb.tile([128, FCW], dt.float32, tag="famp3")
    fqn = sb.tile([128, FCW], dt.float32, tag="fqn")
    fin = sb.tile([128, FCW], dt.float32, tag="fin")
    nc.gpsimd.dma_start(xi[:], x_d[0:1, :].rearrange("o (p c) -> (o p) c", p=128))
    nc.gpsimd.dma_start(xq[:], x_d[1:2, :].rearrange("o (p c) -> (o p) c", p=128))
    nc.vector.tensor_tensor(fs[:], xi[:], xi[:], Alu.mult)
    nc.vector.tensor_tensor(fv[:], xq[:], xq[:], Alu.mult)
    nc.vector.tensor_tensor(fs[:], fs[:], fv[:], Alu.add)
    nc.scalar.activation(fv[:], fs[:], Act.Abs_reciprocal_sqrt)
    nc.vector.tensor_tensor(famp[:], fs[:], fv[:], Alu.mult)
    nc.vector.tensor_tensor(famp3[:], fs[:], famp[:], Alu.mult)
    nc.vector.tensor_tensor(fqn[:], xq[:], fv[:], Alu.mult)
    nc.vector.tensor_tensor(fin[:], xi[:], fv[:], Alu.mult)
    nc.vector.memset(feat[:], 0.0)
    for f, src_t in enumerate((xi, xq, famp, famp3, fqn, fin)):
        nc.gpsimd.dma_start(feat[f:f + 1, :], src_t[:])

    def seq_init():
        nc.vector.memset(h[:], 0.0)
        nc.vector.memset(hp[:], 0.0)
        nc.vector.memset(xp1[:], 0.0)
        nc.vector.memset(xp0[:], 0.0)
        for l in range(2):
            for m in range(4):
                nc.tensor.matmul(rz_ps(l, 0, m), bias_ap(bo[("rz", l)] + m * 128, 128), ones, start=(m == 0), stop=False)
                nc.tensor.matmul(rz_ps(l, 1, m), bias_ap(bo[("rz", l)] + H + m * 128, 128), ones, start=False, stop=False)
                nc.tensor.matmul(n_ps(l, 0, m), bias_ap(bo[("n", l)] + m * 128, 128), ones, start=(m == 0), stop=False)
                nc.tensor.matmul(n_ps(l, 1, m), bias_ap(bo[("nh", l)] + m * 128, 128), ones, start=False, stop=False)

    def delta_block(d_out, scr, cur, prev, th):
        nc.vector.tensor_tensor(scr, cur, prev, Alu.subtract)
        nc.vector.tensor_tensor(d_out, scr, scr, Alu.mult)
        nc.vector.scalar_tensor_tensor(d_out, d_out, th * th, scr, Alu.is_ge, Alu.mult)
        nc.vector.tensor_tensor(prev, prev, d_out, Alu.add)

    def split_hi_lo(src_f32, hi_t, lo_t):
        nc.vector.tensor_copy(hi_t, src_f32)
        nc.vector.tensor_tensor(lo_t, src_f32, hi_t, Alu.subtract)

    def mm3(out_ps, key, rhs_hi, rhs_lo, l):
        whi, wlo = wtb[key]
        if l == 0 and key[0] == "ih":
            whi = whi[0:6, :]; wlo = wlo[0:6, :]
        nc.tensor.matmul(out_ps, whi, rhs_hi, start=False, stop=False)
        nc.tensor.matmul(out_ps, whi, rhs_lo, start=False, stop=False)
        nc.tensor.matmul(out_ps, wlo, rhs_hi, start=False, stop=False)

    def layer_mms_split(l, rhs_hi_ks, rhs_lo_ks, rhs_in_hi, rhs_in_lo, kin):
        for m in range(4):
            for g, gi in (("r", 0), ("z", 1)):
                for k in range(KT):
                    mm3(rz_ps(l, gi, m), ("hh", l, g, m, k), rhs_hi_ks(k), rhs_lo_ks(k), l)
            for k in range(KT):
                mm3(n_ps(l, 1, m), ("hh", l, "nh", m, k), rhs_hi_ks(k), rhs_lo_ks(k), l)
            for g, gi in (("r", 0), ("z", 1)):
                for k in range(kin):
                    mm3(rz_ps(l, gi, m), ("ih", l, g, m, k), rhs_in_hi(k), rhs_in_lo(k), l)
            for k in range(kin):
                mm3(n_ps(l, 0, m), ("ih", l, "n", m, k), rhs_in_hi(k), rhs_in_lo(k), l)

    def gates(l):
        nc.scalar.activation(rz_s[:], ps_rz[l][:], Act.Sigmoid)
        rpart = rz_s[:, 0:KT * bc]
        zpart = rz_s[:, KT * bc:2 * KT * bc]
        nc.vector.tensor_tensor(a_s[:], rpart, ps_n[l][:, KT * bc:2 * KT * bc], Alu.mult)
        nc.vector.tensor_tensor(b_s[:], ps_n[l][:, 0:KT * bc], a_s[:], Alu.add)
        nc.scalar.activation(n_s[:], b_s[:], Act.Tanh)
        nc.vector.tensor_tensor(u_s[:], hsl(l), n_s[:], Alu.subtract)
        nc.vector.tensor_tensor(u_s[:], zpart, u_s[:], Alu.mult)
        nc.vector.tensor_tensor(hsl(l), n_s[:], u_s[:], Alu.add)

    def tick_loop():
        seq_init()
        with tc.For_i(0, T, 1, hint_engines=(mybir.EngineType.PE, mybir.EngineType.DVE)) as iv:
            xt8 = feat[0:8, bass.ds(iv * bc, bc)]
            delta_block(dh[:], sc[:], h[:], hp[:], TH_H)
            delta_block(dx0[:], sc0[:], xt8, xp0[:], TH_X)
            split_hi_lo(dh[:], dh_hi[:], dh_lo[:])
            split_hi_lo(dx0[:], dx0_hi[:], dx0_lo[:])
            layer_mms_split(0,
                            lambda k: dh_hi[:, k * bc:(k + 1) * bc],
                            lambda k: dh_lo[:, k * bc:(k + 1) * bc],
                            lambda k: dx0_hi[0:6, :], lambda k: dx0_lo[0:6, :], 1)
            gates(0)
            delta_block(dx1[:], sc1[:], hsl(0), xp1[:], TH_X)
            split_hi_lo(dx1[:], dx1_hi[:], dx1_lo[:])
            layer_mms_split(1,
                            lambda k: dh_hi[:, (KT + k) * bc:(KT + k + 1) * bc],
                            lambda k: dh_lo[:, (KT + k) * bc:(KT + k + 1) * bc],
                            lambda k: dx1_hi[:, k * bc:(k + 1) * bc],
                            lambda k: dx1_lo[:, k * bc:(k + 1) * bc], KT)
            gates(1)
            nc.tensor.matmul(ps_fc[:], bias_ap(bo["fc"], 2), ones, start=True, stop=False)
            for k in range(KT):
                nc.tensor.matmul(ps_fc[:], fc_tiles[:, k * 2:(k + 1) * 2], hks(1, k), start=False, stop=(k == KT - 1))
            nc.vector.tensor_copy(outring[:, bass.ds(iv * bc, bc)], ps_fc[:])

    if reps == 1:
        tick_loop()
    else:
        with tc.For_i(0, reps, 1):
            tick_loop()

    nc.gpsimd.dma_start(o_d[:, :], outring[:])


_NC_CACHE = {}


def kernel(**inputs) -> np.ndarray:
    x = np.asarray(inputs["x"], np.float32)            # [64, 1024, 2]
    wblob = _pack_weights_fp(inputs, BC)
    wbblob = _pack_weights_bf(inputs)
    if ("k", T, BC) not in _NC_CACHE:
        _NC_CACHE[("k", T, BC)] = _build_kernel(T, BC)
    nc = _NC_CACHE[("k", T, BC)]

    in_maps = []
    for c in range(NCORES):
        xs = x[c * BC:(c + 1) * BC]                    # [bc, T, 2]
        xin = np.ascontiguousarray(xs.transpose(2, 1, 0).reshape(2, T * BC))
        in_maps.append({"xin": xin, "wblob": wblob, "wbblob": wbblob})

    res = run_bass_kernel_spmd(nc, in_maps, core_ids=list(range(NCORES)))
    outs = []
    for c in range(NCORES):
        o = res.results[c]["out"]                      # [2, T*bc]
        outs.append(np.ascontiguousarray(o.reshape(2, T, BC).transpose(2, 1, 0)))
    return np.concatenate(outs, axis=0).astype(np.float32)



# revision 6
# speedup vs baseline: 1.0190x; 1.0190x over previous
"""DeltaGRU Trainium2 kernel: 2-layer delta-GRU (H=512) over T=1024, B=64.

Strategy: data-parallel over batch across 8 NeuronCores (8 samples/core),
weights replicated. Per core one Bass kernel runs the whole recurrence:

- State kept H-major ("transposed": H on partitions, batch on free dim) so all
  gate math runs on [128, *] tiles.
- Matmuls: mac.T = W @ delta with weight tiles stationary as lhsT. Weights and
  per-tick deltas are split hi/lo into bf16 pairs; each logical fp32 matmul is
  3 bf16 matmuls (Whi@dhi + Whi@dlo + Wlo@dhi) accumulating in fp32 PSUM.
  Per-element error ~5e-6, far below this problem's intrinsic fp32 chaos floor
  (threshold flips amplify any rounding to ~1e-2 maxrel), and ~6x faster than
  fp32 matmuls whose 4-byte stationary-weight load dominates.
- The GRU delta-memory (dm, dm_nh) lives permanently in PSUM; each tick's
  matmuls accumulate into it (start=False); sigmoid/tanh read PSUM directly.
  One PSUM bank is claimed once (start=True) by the bias-init matmuls.
- Input feature expansion (i, q, amp, amp^3, q/amp, i/amp) computed on device
  in a pre-pass; per-tick input slices are dynamic SBUF views - the recurrent
  loop performs zero DMA.
- One For_i dynamic loop over the 1024 timesteps; sigmoid+tanh share one ACT
  table so there are no activation-table reloads inside the loop.

kernel(**inputs) takes the full unsharded inputs, returns [64, 1024, 2] f32.
"""
import numpy as np
import ml_dtypes
import concourse.bass as bass
import concourse.tile as tile
import concourse.mybir as mybir
from concourse import bacc
from concourse.bass_utils import run_bass_kernel_spmd

dt = mybir.dt
Alu = mybir.AluOpType
Act = mybir.ActivationFunctionType

H = 512
KT = H // 128
TH_X = 0.1
TH_H = 0.05
B, T, OUT = 64, 1024, 2
NCORES = 8
BC = B // NCORES


def _bias_layout(bc):
    bo = {}
    cur = 0
    for l in range(2):
        bo[("rz", l)] = cur; cur += 2 * H
        bo[("n", l)] = cur; cur += H
        bo[("nh", l)] = cur; cur += H
    bo["fc"] = cur; cur += 2
    bo["ones"] = cur; cur += bc
    return bo, cur


def _weight_mats(inp):
    return (
        np.ascontiguousarray(np.asarray(inp["W_hh_l0"], np.float32).T),
        np.ascontiguousarray(np.asarray(inp["W_ih_l0"], np.float32).T),
        np.ascontiguousarray(np.asarray(inp["W_ih_l1"], np.float32).T),
        np.ascontiguousarray(np.asarray(inp["W_hh_l1"], np.float32).T),
    )


def _pack_weights_fp(inp, bc):
    """fp32 blob: FC tiles + bias/ones row (hidden weights live in the bf16 blob)."""
    cols = []
    Wfc = np.ascontiguousarray(np.asarray(inp["W_fc"], np.float32).T)
    fc = np.zeros((128, 4 * 2), np.float32)
    for k in range(KT):
        fc[:, k * 2:(k + 1) * 2] = Wfc[k * 128:(k + 1) * 128, :]
    cols.append(fc)

    bo, blen = _bias_layout(bc)
    brow = np.zeros((blen,), np.float32)
    for l in range(2):
        b_ih = np.asarray(inp[f"b_ih_l{l}"], np.float32)
        b_hh = np.asarray(inp[f"b_hh_l{l}"], np.float32)
        brow[bo[("rz", l)]:bo[("rz", l)] + 2 * H] = np.concatenate(
            [b_ih[:H] + b_hh[:H], b_ih[H:2 * H] + b_hh[H:2 * H]])
        brow[bo[("n", l)]:bo[("n", l)] + H] = b_ih[2 * H:]
        brow[bo[("nh", l)]:bo[("nh", l)] + H] = b_hh[2 * H:]
    brow[bo["fc"]:bo["fc"] + 2] = np.asarray(inp["b_fc"], np.float32)
    brow[bo["ones"]:bo["ones"] + bc] = 1.0
    bt = np.zeros((128, blen), np.float32)
    bt[0, :] = brow
    cols.append(bt)
    return np.ascontiguousarray(np.concatenate(cols, axis=1))


def _pack_weights_bf(inp):
    """bf16 hi/lo tile blob; order mirrors _emit's wtb bookkeeping."""
    cols = []

    def tile_pair(mat_T, k0, kn, m0):
        t = np.zeros((128, 128), np.float32)
        t[:kn, :] = mat_T[k0:k0 + kn, m0:m0 + 128]
        hi = t.astype(ml_dtypes.bfloat16)
        lo = (t - hi.astype(np.float32)).astype(ml_dtypes.bfloat16)
        return hi, lo

    Whh0, Wih0, Wih1, Whh1 = _weight_mats(inp)
    for W in (Whh0, Whh1):
        for row0 in (0, H, 2 * H):
            for m in range(4):
                for k in range(KT):
                    hi, lo = tile_pair(W, k * 128, 128, row0 + m * 128)
                    cols.append(hi); cols.append(lo)
    for row0 in (0, H, 2 * H):
        for m in range(4):
            hi, lo = tile_pair(Wih0, 0, 6, row0 + m * 128)
            cols.append(hi); cols.append(lo)
    for row0 in (0, H, 2 * H):
        for m in range(4):
            for k in range(KT):
                hi, lo = tile_pair(Wih1, k * 128, 128, row0 + m * 128)
                cols.append(hi); cols.append(lo)
    return np.ascontiguousarray(np.concatenate(cols, axis=1))


def _wcols_fp(bc):
    _, blen = _bias_layout(bc)
    return 8 + blen


def _wbcols():
    return (2 * 3 * 4 * KT + 3 * 4 + 3 * 4 * KT) * 2 * 128


def _build_kernel(T_, bc, reps=1):
    nc = bacc.Bacc("TRN2", target_bir_lowering=False)
    x_d = nc.dram_tensor("xin", [2, T_ * bc], dt.float32, kind="ExternalInput")
    w_d = nc.dram_tensor("wblob", [128, _wcols_fp(bc)], dt.float32, kind="ExternalInput")
    wb_d = nc.dram_tensor("wbblob", [128, _wbcols()], dt.bfloat16, kind="ExternalInput")
    o_d = nc.dram_tensor("out", [2, T_ * bc], dt.float32, kind="ExternalOutput")
    with tile.TileContext(nc) as tc:
        _emit(nc, tc, x_d, w_d, wb_d, o_d, T_, bc, reps)
    nc.finalize()
    return nc


UNROLL = 1


def _emit(nc, tc, x_d, w_d, wb_d, o_d, T, bc, reps=1):
    import contextlib
    ctx = contextlib.ExitStack()
    sb = ctx.enter_context(tc.tile_pool(name="sb", bufs=1))
    ps = ctx.enter_context(tc.tile_pool(name="ps", bufs=1, space="PSUM"))

    w_s = sb.tile([128, _wcols_fp(bc)], dt.float32, tag="wblob")
    nc.gpsimd.dma_start(w_s[:], w_d[:, :])
    wb_s = sb.tile([128, _wbcols()], dt.bfloat16, tag="wbblob")
    nc.gpsimd.dma_start(wb_s[:], wb_d[:, :])
    feat = sb.tile([8, T * bc], dt.float32, tag="feat")

    fc_tiles = w_s[:, 0:8]
    bias_off = 8
    bo, _ = _bias_layout(bc)

    def bias_ap(start, ln):
        return w_s[0:1, bias_off + start: bias_off + start + ln]

    ones = bias_ap(bo["ones"], bc)

    wtb = {}
    boff = [0]

    def next_btile():
        ap = wb_s[:, boff[0]:boff[0] + 128]
        boff[0] += 128
        return ap

    for l in range(2):
        for g in ("r", "z", "nh"):
            for m in range(4):
                for k in range(KT):
                    wtb[("hh", l, g, m, k)] = (next_btile(), next_btile())
    for g in ("r", "z", "n"):
        for m in range(4):
            wtb[("ih", 0, g, m, 0)] = (next_btile(), next_btile())
    for g in ("r", "z", "n"):
        for m in range(4):
            for k in range(KT):
                wtb[("ih", 1, g, m, k)] = (next_btile(), next_btile())

    h = sb.tile([128, 2 * KT * bc], dt.float32, tag="h")
    hp = sb.tile([128, 2 * KT * bc], dt.float32, tag="hp")
    dh = sb.tile([128, 2 * KT * bc], dt.float32, tag="dh")
    sc = sb.tile([128, 2 * KT * bc], dt.float32, tag="sc")
    xp1 = sb.tile([128, KT * bc], dt.float32, tag="xp1")
    dx1 = sb.tile([128, KT * bc], dt.float32, tag="dx1")
    sc1 = sb.tile([128, KT * bc], dt.float32, tag="sc1")
    xp0 = sb.tile([8, bc], dt.float32, tag="xp0")
    dx0 = sb.tile([8, bc], dt.float32, tag="dx0")
    sc0 = sb.tile([8, bc], dt.float32, tag="sc0")
    rz_s = sb.tile([128, 8 * bc], dt.float32, tag="rzs")
    a_s = sb.tile([128, KT * bc], dt.float32, tag="as")
    b_s = sb.tile([128, KT * bc], dt.float32, tag="bs")
    n_s = sb.tile([128, KT * bc], dt.float32, tag="ns")
    u_s = sb.tile([128, KT * bc], dt.float32, tag="us")
    outring = sb.tile([2, T * bc], dt.float32, tag="outring")
    dh_hi = sb.tile([128, 2 * KT * bc], dt.bfloat16, tag="dhhi")
    dh_lo = sb.tile([128, 2 * KT * bc], dt.bfloat16, tag="dhlo")
    dx1_hi = sb.tile([128, KT * bc], dt.bfloat16, tag="dx1hi")
    dx1_lo = sb.tile([128, KT * bc], dt.bfloat16, tag="dx1lo")
    dx0_hi = sb.tile([8, bc], dt.bfloat16, tag="dx0hi")
    dx0_lo = sb.tile([8, bc], dt.bfloat16, tag="dx0lo")

    def hsl(l):
        return h[:, l * KT * bc:(l + 1) * KT * bc]

    def hks(l, k):
        return h[:, (l * KT + k) * bc:(l * KT + k + 1) * bc]

    ps_rz = []
    for l in range(2):
        t = ps.tile([128, 8 * bc], dt.float32, tag=f"psrz{l}", name=f"psrz{l}")
        ps_rz.append(t)
    ps_n = []
    for l in range(2):
        t = ps.tile([128, 8 * bc], dt.float32, tag=f"psn{l}", name=f"psn{l}")
        ps_n.append(t)
    ps_fc = ps.tile([2, bc], dt.float32, tag="psfc")

    def rz_ps(l, g, m):
        return ps_rz[l][:, (g * 4 + m) * bc:(g * 4 + m + 1) * bc]

    def n_ps(l, g, m):
        return ps_n[l][:, (g * 4 + m) * bc:(g * 4 + m + 1) * bc]

    # feature expansion on [128, N/128] tiles, scattered into feat rows via DMA
    N = T * bc
    FCW = N // 128
    xi = sb.tile([128, FCW], dt.float32, tag="xi")
    xq = sb.tile([128, FCW], dt.float32, tag="xq")
    fs = sb.tile([128, FCW], dt.float32, tag="fs")
    fv = sb.tile([128, FCW], dt.float32, tag="fv")
    famp = sb.tile([128, FCW], dt.float32, tag="famp")
    famp3 = sb.tile([128, FCW], dt.float32, tag="famp3")
    fqn = sb.tile([128, FCW], dt.float32, tag="fqn")
    fin = sb.tile([128, FCW], dt.float32, tag="fin")
    nc.gpsimd.dma_start(xi[:], x_d[0:1, :].rearrange("o (p c) -> (o p) c", p=128))
    nc.gpsimd.dma_start(xq[:], x_d[1:2, :].rearrange("o (p c) -> (o p) c", p=128))
    nc.vector.tensor_tensor(fs[:], xi[:], xi[:], Alu.mult)
    nc.vector.tensor_tensor(fv[:], xq[:], xq[:], Alu.mult)
    nc.vector.tensor_tensor(fs[:], fs[:], fv[:], Alu.add)
    nc.scalar.activation(fv[:], fs[:], Act.Abs_reciprocal_sqrt)
    nc.vector.tensor_tensor(famp[:], fs[:], fv[:], Alu.mult)
    nc.vector.tensor_tensor(famp3[:], fs[:], famp[:], Alu.mult)
    nc.vector.tensor_tensor(fqn[:], xq[:], fv[:], Alu.mult)
    nc.vector.tensor_tensor(fin[:], xi[:], fv[:], Alu.mult)
    nc.vector.memset(feat[:], 0.0)
    for f, src_t in enumerate((xi, xq, famp, famp3, fqn, fin)):
        nc.gpsimd.dma_start(feat[f:f + 1, :], src_t[:])

    def seq_init():
        nc.vector.memset(h[:], 0.0)
        nc.vector.memset(hp[:], 0.0)
        nc.vector.memset(xp1[:], 0.0)
        nc.vector.memset(xp0[:], 0.0)
        for l in range(2):
            for m in range(4):
                nc.tensor.matmul(rz_ps(l, 0, m), bias_ap(bo[("rz", l)] + m * 128, 128), ones, start=(m == 0), stop=False)
                nc.tensor.matmul(rz_ps(l, 1, m), bias_ap(bo[("rz", l)] + H + m * 128, 128), ones, start=False, stop=False)
                nc.tensor.matmul(n_ps(l, 0, m), bias_ap(bo[("n", l)] + m * 128, 128), ones, start=(m == 0), stop=False)
                nc.tensor.matmul(n_ps(l, 1, m), bias_ap(bo[("nh", l)] + m * 128, 128), ones, start=False, stop=False)

    def delta_block(d_out, scr, cur, prev, th):
        nc.vector.tensor_tensor(scr, cur, prev, Alu.subtract)
        nc.vector.tensor_tensor(d_out, scr, scr, Alu.mult)
        nc.vector.scalar_tensor_tensor(d_out, d_out, th * th, scr, Alu.is_ge, Alu.mult)
        nc.vector.tensor_tensor(prev, prev, d_out, Alu.add)

    def split_hi_lo(src_f32, hi_t, lo_t):
        nc.vector.tensor_copy(hi_t, src_f32)
        nc.vector.tensor_tensor(lo_t, src_f32, hi_t, Alu.subtract)

    def mm3(out_ps, key, rhs_hi, rhs_lo, l):
        whi, wlo = wtb[key]
        if l == 0 and key[0] == "ih":
            whi = whi[0:6, :]; wlo = wlo[0:6, :]
        nc.tensor.matmul(out_ps, whi, rhs_hi, start=False, stop=False)
        nc.tensor.matmul(out_ps, whi, rhs_lo, start=False, stop=False)
        nc.tensor.matmul(out_ps, wlo, rhs_hi, start=False, stop=False)

    def layer_mms_split(l, rhs_hi_ks, rhs_lo_ks, rhs_in_hi, rhs_in_lo, kin):
        for m in range(4):
            for g, gi in (("r", 0), ("z", 1)):
                for k in range(KT):
                    mm3(rz_ps(l, gi, m), ("hh", l, g, m, k), rhs_hi_ks(k), rhs_lo_ks(k), l)
            for k in range(KT):
                mm3(n_ps(l, 1, m), ("hh", l, "nh", m, k), rhs_hi_ks(k), rhs_lo_ks(k), l)
            for g, gi in (("r", 0), ("z", 1)):
                for k in range(kin):
                    mm3(rz_ps(l, gi, m), ("ih", l, g, m, k), rhs_in_hi(k), rhs_in_lo(k), l)
            for k in range(kin):
                mm3(n_ps(l, 0, m), ("ih", l, "n", m, k), rhs_in_hi(k), rhs_in_lo(k), l)

    def gates(l):
        nc.scalar.activation(rz_s[:], ps_rz[l][:], Act.Sigmoid)
        rpart = rz_s[:, 0:KT * bc]
        zpart = rz_s[:, KT * bc:2 * KT * bc]
        nc.vector.tensor_tensor(a_s[:], rpart, ps_n[l][:, KT * bc:2 * KT * bc], Alu.mult)
        nc.vector.tensor_tensor(b_s[:], ps_n[l][:, 0:KT * bc], a_s[:], Alu.add)
        nc.scalar.activation(n_s[:], b_s[:], Act.Tanh)
        nc.vector.tensor_tensor(u_s[:], hsl(l), n_s[:], Alu.subtract)
        nc.vector.tensor_tensor(u_s[:], zpart, u_s[:], Alu.mult)
        nc.vector.tensor_tensor(hsl(l), n_s[:], u_s[:], Alu.add)

    def tick_body(off):
        xt8 = feat[0:8, bass.ds(off, bc)]
        delta_block(dh[:], sc[:], h[:], hp[:], TH_H)
        delta_block(dx0[:], sc0[:], xt8, xp0[:], TH_X)
        split_hi_lo(dh[:], dh_hi[:], dh_lo[:])
        split_hi_lo(dx0[:], dx0_hi[:], dx0_lo[:])
        layer_mms_split(0,
                        lambda k: dh_hi[:, k * bc:(k + 1) * bc],
                        lambda k: dh_lo[:, k * bc:(k + 1) * bc],
                        lambda k: dx0_hi[0:6, :], lambda k: dx0_lo[0:6, :], 1)
        gates(0)
        delta_block(dx1[:], sc1[:], hsl(0), xp1[:], TH_X)
        split_hi_lo(dx1[:], dx1_hi[:], dx1_lo[:])
        layer_mms_split(1,
                        lambda k: dh_hi[:, (KT + k) * bc:(KT + k + 1) * bc],
                        lambda k: dh_lo[:, (KT + k) * bc:(KT + k + 1) * bc],
                        lambda k: dx1_hi[:, k * bc:(k + 1) * bc],
                        lambda k: dx1_lo[:, k * bc:(k + 1) * bc], KT)
        gates(1)
        nc.tensor.matmul(ps_fc[:], bias_ap(bo["fc"], 2), ones, start=True, stop=False)
        for k in range(KT):
            nc.tensor.matmul(ps_fc[:], fc_tiles[:, k * 2:(k + 1) * 2], hks(1, k), start=False, stop=(k == KT - 1))
        nc.vector.tensor_copy(outring[:, bass.ds(off, bc)], ps_fc[:])

    def tick_loop():
        seq_init()
        U = UNROLL
        with tc.For_i(0, T, U, hint_engines=(mybir.EngineType.PE, mybir.EngineType.DVE)) as iv:
            for u in range(U):
                tick_body(iv * bc + u * bc)

    if reps == 1:
        tick_loop()
    else:
        with tc.For_i(0, reps, 1):
            tick_loop()

    nc.gpsimd.dma_start(o_d[:, :], outring[:])


_NC_CACHE = {}


def kernel(**inputs) -> np.ndarray:
    x = np.asarray(inputs["x"], np.float32)            # [64, 1024, 2]
    wblob = _pack_weights_fp(inputs, BC)
    wbblob = _pack_weights_bf(inputs)
    if ("k", T, BC) not in _NC_CACHE:
        _NC_CACHE[("k", T, BC)] = _build_kernel(T, BC)
    nc = _NC_CACHE[("k", T, BC)]

    in_maps = []
    for c in range(NCORES):
        xs = x[c * BC:(c + 1) * BC]                    # [bc, T, 2]
        xin = np.ascontiguousarray(xs.transpose(2, 1, 0).reshape(2, T * BC))
        in_maps.append({"xin": xin, "wblob": wblob, "wbblob": wbblob})

    res = run_bass_kernel_spmd(nc, in_maps, core_ids=list(range(NCORES)))
    outs = []
    for c in range(NCORES):
        o = res.results[c]["out"]                      # [2, T*bc]
        outs.append(np.ascontiguousarray(o.reshape(2, T, BC).transpose(2, 1, 0)))
    return np.concatenate(outs, axis=0).astype(np.float32)



# revision 8
# speedup vs baseline: 1.3095x; 1.2851x over previous
"""DeltaGRU Trainium2 kernel: 2-layer delta-GRU (H=512) over T=1024, B=64.

Strategy: data-parallel over batch across 8 NeuronCores (8 samples/core),
weights replicated. Per core one Bass kernel runs the whole recurrence:

- State kept H-major ("transposed": H on partitions, batch on free dim) so all
  gate math runs on [128, *] tiles.
- Matmuls: mac.T = W @ delta with weight tiles stationary as lhsT. Weights and
  per-tick deltas are split hi/lo into bf16 pairs; each logical fp32 matmul is
  3 bf16 matmuls (Whi@dhi + Whi@dlo + Wlo@dhi) accumulating in fp32 PSUM.
  Per-element error ~5e-6, far below this problem's intrinsic fp32 chaos floor
  (threshold flips amplify any rounding to ~1e-2 maxrel), and ~6x faster than
  fp32 matmuls whose 4-byte stationary-weight load dominates.
- The GRU delta-memory (dm, dm_nh) lives permanently in PSUM; each tick's
  matmuls accumulate into it (start=False); sigmoid/tanh read PSUM directly.
  One PSUM bank is claimed once (start=True) by the bias-init matmuls.
- Input feature expansion (i, q, amp, amp^3, q/amp, i/amp) computed on device
  in a pre-pass; per-tick input slices are dynamic SBUF views - the recurrent
  loop performs zero DMA.
- One For_i dynamic loop over the 1024 timesteps; sigmoid+tanh share one ACT
  table so there are no activation-table reloads inside the loop.

kernel(**inputs) takes the full unsharded inputs, returns [64, 1024, 2] f32.
"""
import numpy as np
import ml_dtypes
import concourse.bass as bass
import concourse.tile as tile
import concourse.mybir as mybir
from concourse import bacc
from concourse.bass_utils import run_bass_kernel_spmd

dt = mybir.dt
Alu = mybir.AluOpType
Act = mybir.ActivationFunctionType

H = 512
KT = H // 128
TH_X = 0.1
TH_H = 0.05
B, T, OUT = 64, 1024, 2
NCORES = 8
BC = B // NCORES


def _bias_layout(bc):
    bo = {}
    cur = 0
    for l in range(2):
        bo[("rz", l)] = cur; cur += 2 * H
        bo[("n", l)] = cur; cur += H
        bo[("nh", l)] = cur; cur += H
    bo["fc"] = cur; cur += 2
    bo["ones"] = cur; cur += bc
    return bo, cur


def _weight_mats(inp):
    return (
        np.ascontiguousarray(np.asarray(inp["W_hh_l0"], np.float32).T),
        np.ascontiguousarray(np.asarray(inp["W_ih_l0"], np.float32).T),
        np.ascontiguousarray(np.asarray(inp["W_ih_l1"], np.float32).T),
        np.ascontiguousarray(np.asarray(inp["W_hh_l1"], np.float32).T),
    )


def _pack_weights_fp(inp, bc):
    """fp32 blob: FC tiles + bias/ones row (hidden weights live in the bf16 blob)."""
    cols = []
    Wfc = np.ascontiguousarray(np.asarray(inp["W_fc"], np.float32).T)
    fc = np.zeros((128, 4 * 2), np.float32)
    for k in range(KT):
        fc[:, k * 2:(k + 1) * 2] = Wfc[k * 128:(k + 1) * 128, :]
    cols.append(fc)

    bo, blen = _bias_layout(bc)
    brow = np.zeros((blen,), np.float32)
    for l in range(2):
        b_ih = np.asarray(inp[f"b_ih_l{l}"], np.float32)
        b_hh = np.asarray(inp[f"b_hh_l{l}"], np.float32)
        brow[bo[("rz", l)]:bo[("rz", l)] + 2 * H] = np.concatenate(
            [b_ih[:H] + b_hh[:H], b_ih[H:2 * H] + b_hh[H:2 * H]])
        brow[bo[("n", l)]:bo[("n", l)] + H] = b_ih[2 * H:]
        brow[bo[("nh", l)]:bo[("nh", l)] + H] = b_hh[2 * H:]
    brow[bo["fc"]:bo["fc"] + 2] = np.asarray(inp["b_fc"], np.float32)
    brow[bo["ones"]:bo["ones"] + bc] = 1.0
    bt = np.zeros((128, blen), np.float32)
    bt[0, :] = brow
    cols.append(bt)
    return np.ascontiguousarray(np.concatenate(cols, axis=1))


def _pack_weights_bf(inp):
    """bf16 hi/lo tile blob; order mirrors _emit's wtb bookkeeping."""
    cols = []

    def tile_pair(mat_T, k0, kn, m0):
        t = np.zeros((128, 128), np.float32)
        t[:kn, :] = mat_T[k0:k0 + kn, m0:m0 + 128]
        hi = t.astype(ml_dtypes.bfloat16)
        lo = (t - hi.astype(np.float32)).astype(ml_dtypes.bfloat16)
        return hi, lo

    Whh0, Wih0, Wih1, Whh1 = _weight_mats(inp)
    for W in (Whh0, Whh1):
        for row0 in (0, H, 2 * H):
            for m in range(4):
                for k in range(KT):
                    hi, lo = tile_pair(W, k * 128, 128, row0 + m * 128)
                    cols.append(hi); cols.append(lo)
    for row0 in (0, H, 2 * H):
        for m in range(4):
            hi, lo = tile_pair(Wih0, 0, 6, row0 + m * 128)
            cols.append(hi); cols.append(lo)
    for row0 in (0, H, 2 * H):
        for m in range(4):
            for k in range(KT):
                hi, lo = tile_pair(Wih1, k * 128, 128, row0 + m * 128)
                cols.append(hi); cols.append(lo)
    return np.ascontiguousarray(np.concatenate(cols, axis=1))


def _wcols_fp(bc):
    _, blen = _bias_layout(bc)
    return 8 + blen


def _wbcols():
    return (2 * 3 * 4 * KT + 3 * 4 + 3 * 4 * KT) * 2 * 128


def _build_kernel(T_, bc, reps=1):
    nc = bacc.Bacc("TRN2", target_bir_lowering=False)
    x_d = nc.dram_tensor("xin", [2, T_ * bc], dt.float32, kind="ExternalInput")
    w_d = nc.dram_tensor("wblob", [128, _wcols_fp(bc)], dt.float32, kind="ExternalInput")
    wb_d = nc.dram_tensor("wbblob", [128, _wbcols()], dt.bfloat16, kind="ExternalInput")
    o_d = nc.dram_tensor("out", [2, T_ * bc], dt.float32, kind="ExternalOutput")
    with tile.TileContext(nc) as tc:
        _emit(nc, tc, x_d, w_d, wb_d, o_d, T_, bc, reps)
    nc.finalize()
    return nc


UNROLL = 1
MM_MODE = 3


def _emit(nc, tc, x_d, w_d, wb_d, o_d, T, bc, reps=1):
    import contextlib
    ctx = contextlib.ExitStack()
    sb = ctx.enter_context(tc.tile_pool(name="sb", bufs=1))
    ps = ctx.enter_context(tc.tile_pool(name="ps", bufs=1, space="PSUM"))

    w_s = sb.tile([128, _wcols_fp(bc)], dt.float32, tag="wblob")
    nc.gpsimd.dma_start(w_s[:], w_d[:, :])
    wb_s = sb.tile([128, _wbcols()], dt.bfloat16, tag="wbblob")
    nc.gpsimd.dma_start(wb_s[:], wb_d[:, :])
    feat = sb.tile([8, T * bc], dt.float32, tag="feat")

    fc_tiles = w_s[:, 0:8]
    bias_off = 8
    bo, _ = _bias_layout(bc)

    def bias_ap(start, ln):
        return w_s[0:1, bias_off + start: bias_off + start + ln]

    ones = bias_ap(bo["ones"], bc)

    wtb = {}
    boff = [0]

    def next_btile():
        ap = wb_s[:, boff[0]:boff[0] + 128]
        boff[0] += 128
        return ap

    for l in range(2):
        for g in ("r", "z", "nh"):
            for m in range(4):
                for k in range(KT):
                    wtb[("hh", l, g, m, k)] = (next_btile(), next_btile())
    for g in ("r", "z", "n"):
        for m in range(4):
            wtb[("ih", 0, g, m, 0)] = (next_btile(), next_btile())
    for g in ("r", "z", "n"):
        for m in range(4):
            for k in range(KT):
                wtb[("ih", 1, g, m, k)] = (next_btile(), next_btile())

    h = sb.tile([128, 2 * KT * bc], dt.float32, tag="h")
    hp = sb.tile([128, 2 * KT * bc], dt.float32, tag="hp")
    dh = sb.tile([128, 2 * KT * bc], dt.float32, tag="dh")
    sc = sb.tile([128, 2 * KT * bc], dt.float32, tag="sc")
    xp1 = sb.tile([128, KT * bc], dt.float32, tag="xp1")
    dx1 = sb.tile([128, KT * bc], dt.float32, tag="dx1")
    sc1 = sb.tile([128, KT * bc], dt.float32, tag="sc1")
    xp0 = sb.tile([8, bc], dt.float32, tag="xp0")
    dx0 = sb.tile([8, bc], dt.float32, tag="dx0")
    sc0 = sb.tile([8, bc], dt.float32, tag="sc0")
    rz_s = sb.tile([128, 8 * bc], dt.float32, tag="rzs")
    a_s = sb.tile([128, KT * bc], dt.float32, tag="as")
    b_s = sb.tile([128, KT * bc], dt.float32, tag="bs")
    n_s = sb.tile([128, KT * bc], dt.float32, tag="ns")
    u_s = sb.tile([128, KT * bc], dt.float32, tag="us")
    outring = sb.tile([2, T * bc], dt.float32, tag="outring")
    dh_hi = sb.tile([128, 2 * KT * bc], dt.bfloat16, tag="dhhi")
    dh_lo = sb.tile([128, 2 * KT * bc], dt.bfloat16, tag="dhlo")
    dx1_hi = sb.tile([128, KT * bc], dt.bfloat16, tag="dx1hi")
    dx1_lo = sb.tile([128, KT * bc], dt.bfloat16, tag="dx1lo")
    dx0_hi = sb.tile([8, bc], dt.bfloat16, tag="dx0hi")
    dx0_lo = sb.tile([8, bc], dt.bfloat16, tag="dx0lo")

    def hsl(l):
        return h[:, l * KT * bc:(l + 1) * KT * bc]

    def hks(l, k):
        return h[:, (l * KT + k) * bc:(l * KT + k + 1) * bc]

    ps_rz = []
    for l in range(2):
        t = ps.tile([128, 8 * bc], dt.float32, tag=f"psrz{l}", name=f"psrz{l}")
        ps_rz.append(t)
    ps_n = []
    for l in range(2):
        t = ps.tile([128, 8 * bc], dt.float32, tag=f"psn{l}", name=f"psn{l}")
        ps_n.append(t)
    ps_fc = ps.tile([2, bc], dt.float32, tag="psfc")

    def rz_ps(l, g, m):
        return ps_rz[l][:, (g * 4 + m) * bc:(g * 4 + m + 1) * bc]

    def n_ps(l, g, m):
        return ps_n[l][:, (g * 4 + m) * bc:(g * 4 + m + 1) * bc]

    # feature expansion on [128, N/128] tiles, scattered into feat rows via DMA
    N = T * bc
    FCW = N // 128
    xi = sb.tile([128, FCW], dt.float32, tag="xi")
    xq = sb.tile([128, FCW], dt.float32, tag="xq")
    fs = sb.tile([128, FCW], dt.float32, tag="fs")
    fv = sb.tile([128, FCW], dt.float32, tag="fv")
    famp = sb.tile([128, FCW], dt.float32, tag="famp")
    famp3 = sb.tile([128, FCW], dt.float32, tag="famp3")
    fqn = sb.tile([128, FCW], dt.float32, tag="fqn")
    fin = sb.tile([128, FCW], dt.float32, tag="fin")
    nc.gpsimd.dma_start(xi[:], x_d[0:1, :].rearrange("o (p c) -> (o p) c", p=128))
    nc.gpsimd.dma_start(xq[:], x_d[1:2, :].rearrange("o (p c) -> (o p) c", p=128))
    nc.vector.tensor_tensor(fs[:], xi[:], xi[:], Alu.mult)
    nc.vector.tensor_tensor(fv[:], xq[:], xq[:], Alu.mult)
    nc.vector.tensor_tensor(fs[:], fs[:], fv[:], Alu.add)
    nc.scalar.activation(fv[:], fs[:], Act.Abs_reciprocal_sqrt)
    nc.vector.tensor_tensor(famp[:], fs[:], fv[:], Alu.mult)
    nc.vector.tensor_tensor(famp3[:], fs[:], famp[:], Alu.mult)
    nc.vector.tensor_tensor(fqn[:], xq[:], fv[:], Alu.mult)
    nc.vector.tensor_tensor(fin[:], xi[:], fv[:], Alu.mult)
    nc.vector.memset(feat[:], 0.0)
    for f, src_t in enumerate((xi, xq, famp, famp3, fqn, fin)):
        nc.gpsimd.dma_start(feat[f:f + 1, :], src_t[:])

    def seq_init():
        nc.vector.memset(h[:], 0.0)
        nc.vector.memset(hp[:], 0.0)
        nc.vector.memset(xp1[:], 0.0)
        nc.vector.memset(xp0[:], 0.0)
        for l in range(2):
            for m in range(4):
                nc.tensor.matmul(rz_ps(l, 0, m), bias_ap(bo[("rz", l)] + m * 128, 128), ones, start=(m == 0), stop=False)
                nc.tensor.matmul(rz_ps(l, 1, m), bias_ap(bo[("rz", l)] + H + m * 128, 128), ones, start=False, stop=False)
                nc.tensor.matmul(n_ps(l, 0, m), bias_ap(bo[("n", l)] + m * 128, 128), ones, start=(m == 0), stop=False)
                nc.tensor.matmul(n_ps(l, 1, m), bias_ap(bo[("nh", l)] + m * 128, 128), ones, start=False, stop=False)

    def delta_block(d_out, scr, cur, prev, th):
        nc.vector.tensor_tensor(scr, cur, prev, Alu.subtract)
        nc.vector.tensor_tensor(d_out, scr, scr, Alu.mult)
        nc.vector.scalar_tensor_tensor(d_out, d_out, th * th, scr, Alu.is_ge, Alu.mult)
        nc.vector.tensor_tensor(prev, prev, d_out, Alu.add)

    def split_hi_lo(src_f32, hi_t, lo_t):
        nc.vector.tensor_copy(hi_t, src_f32)
        nc.vector.tensor_tensor(lo_t, src_f32, hi_t, Alu.subtract)

    def mm3(out_ps, key, rhs_hi, rhs_lo, l):
        whi, wlo = wtb[key]
        if l == 0 and key[0] == "ih":
            whi = whi[0:6, :]; wlo = wlo[0:6, :]
        nc.tensor.matmul(out_ps, whi, rhs_hi, start=False, stop=False)
        if MM_MODE >= 2:
            nc.tensor.matmul(out_ps, whi, rhs_lo, start=False, stop=False)
        if MM_MODE >= 3:
            nc.tensor.matmul(out_ps, wlo, rhs_hi, start=False, stop=False)
        if MM_MODE >= 4:
            nc.tensor.matmul(out_ps, wlo, rhs_lo, start=False, stop=False)

    def layer_mms_split(l, rhs_hi_ks, rhs_lo_ks, rhs_in_hi, rhs_in_lo, kin):
        for m in range(4):
            for g, gi in (("r", 0), ("z", 1)):
                for k in range(KT):
                    mm3(rz_ps(l, gi, m), ("hh", l, g, m, k), rhs_hi_ks(k), rhs_lo_ks(k), l)
            for k in range(KT):
                mm3(n_ps(l, 1, m), ("hh", l, "nh", m, k), rhs_hi_ks(k), rhs_lo_ks(k), l)
            for g, gi in (("r", 0), ("z", 1)):
                for k in range(kin):
                    mm3(rz_ps(l, gi, m), ("ih", l, g, m, k), rhs_in_hi(k), rhs_in_lo(k), l)
            for k in range(kin):
                mm3(n_ps(l, 0, m), ("ih", l, "n", m, k), rhs_in_hi(k), rhs_in_lo(k), l)

    def gates(l):
        nc.scalar.activation(rz_s[:], ps_rz[l][:], Act.Sigmoid)
        rpart = rz_s[:, 0:KT * bc]
        zpart = rz_s[:, KT * bc:2 * KT * bc]
        nc.vector.tensor_tensor(a_s[:], rpart, ps_n[l][:, KT * bc:2 * KT * bc], Alu.mult)
        nc.vector.tensor_tensor(b_s[:], ps_n[l][:, 0:KT * bc], a_s[:], Alu.add)
        nc.scalar.activation(n_s[:], b_s[:], Act.Tanh)
        nc.vector.tensor_tensor(u_s[:], hsl(l), n_s[:], Alu.subtract)
        nc.vector.tensor_tensor(u_s[:], zpart, u_s[:], Alu.mult)
        nc.vector.tensor_tensor(hsl(l), n_s[:], u_s[:], Alu.add)

    def tick_body(off):
        xt8 = feat[0:8, bass.ds(off, bc)]
        delta_block(dh[:], sc[:], h[:], hp[:], TH_H)
        delta_block(dx0[:], sc0[:], xt8, xp0[:], TH_X)
        split_hi_lo(dh[:], dh_hi[:], dh_lo[:])
        split_hi_lo(dx0[:], dx0_hi[:], dx0_lo[:])
        layer_mms_split(0,
                        lambda k: dh_hi[:, k * bc:(k + 1) * bc],
                        lambda k: dh_lo[:, k * bc:(k + 1) * bc],
                        lambda k: dx0_hi[0:6, :], lambda k: dx0_lo[0:6, :], 1)
        gates(0)
        delta_block(dx1[:], sc1[:], hsl(0), xp1[:], TH_X)
        split_hi_lo(dx1[:], dx1_hi[:], dx1_lo[:])
        layer_mms_split(1,
                        lambda k: dh_hi[:, (KT + k) * bc:(KT + k + 1) * bc],
                        lambda k: dh_lo[:, (KT + k) * bc:(KT + k + 1) * bc],
                        lambda k: dx1_hi[:, k * bc:(k + 1) * bc],
                        lambda k: dx1_lo[:, k * bc:(k + 1) * bc], KT)
        gates(1)
        nc.tensor.matmul(ps_fc[:], bias_ap(bo["fc"], 2), ones, start=True, stop=False)
        for k in range(KT):
            nc.tensor.matmul(ps_fc[:], fc_tiles[:, k * 2:(k + 1) * 2], hks(1, k), start=False, stop=(k == KT - 1))
        nc.vector.tensor_copy(outring[:, bass.ds(off, bc)], ps_fc[:])

    def tick_loop():
        seq_init()
        U = UNROLL
        with tc.For_i(0, T, U, hint_engines=(mybir.EngineType.PE, mybir.EngineType.DVE)) as iv:
            for u in range(U):
                tick_body(iv * bc + u * bc)

    if reps == 1:
        tick_loop()
    else:
        with tc.For_i(0, reps, 1):
            tick_loop()

    nc.gpsimd.dma_start(o_d[:, :], outring[:])


_NC_CACHE = {}


def kernel(**inputs) -> np.ndarray:
    x = np.asarray(inputs["x"], np.float32)            # [64, 1024, 2]
    wblob = _pack_weights_fp(inputs, BC)
    wbblob = _pack_weights_bf(inputs)
    if ("k", T, BC) not in _NC_CACHE:
        _NC_CACHE[("k", T, BC)] = _build_kernel(T, BC)
    nc = _NC_CACHE[("k", T, BC)]

    in_maps = []
    for c in range(NCORES):
        xs = x[c * BC:(c + 1) * BC]                    # [bc, T, 2]
        xin = np.ascontiguousarray(xs.transpose(2, 1, 0).reshape(2, T * BC))
        in_maps.append({"xin": xin, "wblob": wblob, "wbblob": wbblob})

    res = run_bass_kernel_spmd(nc, in_maps, core_ids=list(range(NCORES)))
    outs = []
    for c in range(NCORES):
        o = res.results[c]["out"]                      # [2, T*bc]
        outs.append(np.ascontiguousarray(o.reshape(2, T, BC).transpose(2, 1, 0)))
    return np.concatenate(outs, axis=0).astype(np.float32)

